# revision 69
# baseline (speedup 1.0000x reference)
"""MoE block (top-1 routing, shared FFN + per-expert LoRA) on 8 TRN2 NeuronCores.

Strategy: data-parallel over the 8192 tokens (1024 tokens/core), weights
replicated. The reference's dense-then-mask expert loop collapses to:

    logits = x @ gate_W.T + gate_b ; mask8 = (logits == rowmax(logits))
    u      = x @ A_cat.T                 [32, N]   (r-major rows: r*8+e)
    u_m    = u * mask8-replicated-4x
    inter  = relu(x @ wi_W.T + Bcat.T @ u_m + wi_b)   (bf16 matmuls)
    out    = inter @ wo_W.T + wo_b

All in transposed (feature-major) layout on chip; the host pre-tiles every
tensor into flat [128, *] layouts so each DMA is a cheap 2D descriptor set,
and re-transposes the output. Logits are fp32-accurate via 3 bf16 terms
(x16@g16 + dx16@g16 + x16@dg16) so routing matches the fp32 argmax; the
top-1 mask is built without PE transposes (gpsimd cross-partition max +
vector equality + a tiny PE replicate matmul).

Schedule: the PE runs one continuous in-order stream — warm-up fillers
(hold the clock-gate/p-state up while DMAs land), router A-terms chasing
the x chunks, then mm1's k-loops paced by the wi stream. The lora term is
decoupled from mm1's PSUM accumulation (base = x@wiT + wi_b goes to SBUF
in bf16; standalone lora matmuls + vector add/relu finalize inter later),
so the router's mask-chain latency never blocks the PE pipeline. All big
DMAs ride the sync hw-DGE ring ordered by first use; outputs (bf16) drain
on the same ring behind the weight loads.
"""

import numpy as np
import ml_dtypes
from contextlib import ExitStack

import concourse.bass as bass
import concourse.tile as tile
from concourse import bacc, mybir
from concourse.bass_utils import run_bass_kernel_spmd

F32 = mybir.dt.float32
BF16 = mybir.dt.bfloat16
BF = ml_dtypes.bfloat16

B, S, D, F, E, R = 4, 2048, 1024, 4096, 8, 4
NCORES = 8
NT = B * S          # 8192 tokens total
N = NT // NCORES    # 1024 tokens per core
ER = E * R          # 32 lora rows (r-major: row r*8+e)
KD = D // 128       # 8 contraction tiles over D
KF = F // 128       # 32 contraction tiles over F
TH = N // 512       # 2 token halves (matmul moving dim)
P = 128
# router stationary cols: [g16(0:8) | zeros(8:32) | Acat16(32:64) | dg16(64:72)]
# -> psum rows: logits at 0:8 (x@g16 + dx@g16), u at 32:64, x@dg16 at 64:72;
# all reads land on quadrant-aligned partition offsets (0/32/64).
CGW = 72
NWARM = 10          # PE warm-up matmuls (keep HAM clock up during DMA-in)
NFILL = 4           # PE fillers between router A-terms and mm1

Relu = mybir.ActivationFunctionType.Relu
Add = mybir.AluOpType.add
IsEq = mybir.AluOpType.is_equal


def _emit(ctx: ExitStack, tc: tile.TileContext, io: dict):
    nc = tc.nc

    consts = ctx.enter_context(tc.tile_pool(name="consts", bufs=1))
    xpool = ctx.enter_context(tc.tile_pool(name="xpool", bufs=1))
    wipool = ctx.enter_context(tc.tile_pool(name="wipool", bufs=1))
    ipool = ctx.enter_context(tc.tile_pool(name="ipool", bufs=1))
    wop = ctx.enter_context(tc.tile_pool(name="wop", bufs=2))
    rwk = ctx.enter_context(tc.tile_pool(name="rwk", bufs=1))
    outp = ctx.enter_context(tc.tile_pool(name="outp", bufs=3))
    sps = ctx.enter_context(tc.tile_pool(name="sps", bufs=1, space="PSUM"))
    bps = ctx.enter_context(tc.tile_pool(name="bps", bufs=6, space="PSUM"))

    TS = [slice(th * 512, (th + 1) * 512) for th in range(TH)]

    # ---------- input DMAs, all on the sync (hw-DGE) queue, in arrival-
    # priority order: router weights, x (4 chunks for fine-grained deps),
    # lora B / biases, wi per f-tile, wo per d-tile.
    # head = cg | repm(row-padded) | x chunk 0, fetched as ONE first DMA
    HCG = KD * CGW
    head_t = consts.tile([P, HCG + ER + N], BF16, tag="head")
    nc.sync.dma_start(out=head_t, in_=io["head"])
    xt = xpool.tile([P, (KD - 1) * N], BF16, tag="x")
    dxt = xpool.tile([P, KD * N], BF16, tag="dx")
    for j in range(KD - 1):  # one chunk per k-tile: router starts ASAP
        sl = slice(j * N, (j + 1) * N)
        nc.sync.dma_start(out=xt[:, sl], in_=io["xT"][:, sl])
    wiB = []
    for f in range(KF):
        t = wipool.tile([P, KD * P], BF16, tag=f"wi{f}")
        wiB.append(t)
    for f in range(3):
        nc.sync.dma_start(out=wiB[f], in_=io["wiB"][f])
    biases_sb = consts.tile([P, 41], F32, tag="biases")
    nc.sync.dma_start(out=biases_sb, in_=io["biases"])
    # dx is only needed once mm1 f0's k-loop is already running
    for j in range(4):
        sl = slice(j * 2 * N, (j + 1) * 2 * N)
        nc.sync.dma_start(out=dxt[:, sl], in_=io["dxT"][:, sl])
    bcat_sb = consts.tile([ER, F], BF16, tag="bc")
    nc.sync.dma_start(out=bcat_sb, in_=io["bT"])
    for f in range(3, KF):
        nc.sync.dma_start(out=wiB[f], in_=io["wiB"][f])
    repm_sb = head_t[0:E, HCG:HCG + ER]
    wo_tiles = []
    for d in range(KD):
        t = wop.tile([P, F], BF16, tag="wo", name=f"wo{d}")
        nc.sync.dma_start(out=t, in_=io["woTt"][d])
        wo_tiles.append(t)

    cg_sb = [head_t[:, k * CGW:(k + 1) * CGW] for k in range(KD)]
    x16 = [head_t[:, HCG + ER:HCG + ER + N]] + \
          [xt[:, k * N:(k + 1) * N] for k in range(KD - 1)]
    dx16 = [dxt[:, k * N:(k + 1) * N] for k in range(KD)]
    wib_sb = biases_sb[:, 0:KF]
    wob_sb = biases_sb[:, KF:KF + KD]
    gateb_col = biases_sb[:, 40:41]

    # ---------- PE warm-up: results never consumed; spins the PE from
    # engine-boot, holding the HAM clock gate open while the input DMAs land.
    warm_src = consts.tile([P, 512], BF16, tag="warm")
    nc.gpsimd.memset(warm_src, 1.0)
    ones_row = consts.tile([1, 64], BF16, tag="ones_row")
    nc.vector.memset(ones_row, 1.0)
    for w in range(NWARM):
        pw = bps.tile([P, 512], F32, tag="pbig", name=f"pw{w}")
        nc.tensor.matmul(pw, lhsT=warm_src[:, 0:P], rhs=warm_src,
                         start=True, stop=True)

    # ---------- router + lora projection (fp32-accurate logits via 3 bf16
    # terms: x16@g16 + dx16@g16 + x16@dg16; u needs no correction).
    # A-terms chase the x chunks (with fillers bridging DMA-arrival waits);
    # the dx B-terms are deferred to overlap with mm1's first k-loop, since
    # only the bcat stop (not the k-loops) depends on the router.
    pcu = [sps.tile([CGW, 512], F32, tag=f"pcu{th}", name=f"pcu{th}")
           for th in range(TH)]
    nfil = 0
    for k in range(KD):
        for th in range(TH):
            nc.tensor.matmul(pcu[th], lhsT=cg_sb[k], rhs=x16[k][:, TS[th]],
                             start=(k == 0), stop=False)
        # fillers bridge the DMA arrival rate (slower during DGE ramp-up)
        for _ in range(2 if 0 < k < 4 else 1):
            pw = bps.tile([P, 512], F32, tag="pbig", name=f"pf{nfil}")
            nc.tensor.matmul(pw, lhsT=warm_src[:, 0:P], rhs=warm_src,
                             start=True, stop=True)
            nfil += 1
    for w in range(NFILL):
        pw = bps.tile([P, 512], F32, tag="pbig", name=f"pf{nfil + w}")
        nc.tensor.matmul(pw, lhsT=warm_src[:, 0:P], rhs=warm_src,
                         start=True, stop=True)

    um16, lgb, mask8 = [], [], []
    for th in range(TH):
        lgb.append(rwk.tile([E, 512], F32, tag=f"lgb{th}", name=f"lgb{th}"))
        mask8.append(rwk.tile([E, 512], BF16, tag=f"mk{th}", name=f"mk{th}"))
        um16.append(rwk.tile([ER, 512], BF16, tag=f"um{th}", name=f"um{th}"))
    mx8 = [rwk.tile([E, 512], F32, tag=f"mx8_{th}", name=f"mx8_{th}")
           for th in range(TH)]
    lgc = [rwk.tile([E, 512], F32, tag=f"lgc{th}", name=f"lgc{th}")
           for th in range(TH)]

    inter_sb = [ipool.tile([P, N], BF16, tag=f"inter{f}", name=f"inter{f}")
                for f in range(KF)]

    # mm1 is decoupled from the router: each f writes base = x@wiT + wi_b to
    # SBUF (bf16, Identity act). The lora term is added later by standalone
    # matmuls + a vector add/relu pass, so the router's mask-chain latency
    # never blocks the PE pipeline.
    def kloop(f):
        ps = [bps.tile([P, 512], F32, tag="pbig", name=f"p1_{f}_{th}")
              for th in range(TH)]
        for k in range(KD):
            for th in range(TH):
                nc.tensor.matmul(ps[th], lhsT=wiB[f][:, k * P:(k + 1) * P],
                                 rhs=x16[k][:, TS[th]],
                                 start=(k == 0), stop=(k == KD - 1))
        for th in range(TH):
            nc.scalar.activation(inter_sb[f][:, TS[th]], ps[th],
                                 mybir.ActivationFunctionType.Identity,
                                 bias=wib_sb[:, f:f + 1])
        return ps

    def lora(f):
        # vector does the PSUM-reading add; gpsimd (no PSUM access) the relu
        for th in range(TH):
            lp = bps.tile([P, 512], F32, tag="pbig", name=f"lp{f}_{th}")
            nc.tensor.matmul(lp[0:P, :], lhsT=bcat_sb[:, f * P:(f + 1) * P],
                             rhs=um16[th], start=True, stop=True)
            sl = inter_sb[f][:, TS[th]]
            nc.vector.scalar_tensor_tensor(sl, lp, 0.0, sl, Add, Add)
            nc.vector.tensor_scalar(sl, sl, 0.0, None, mybir.AluOpType.max)

    kloop(0)
    # dx B-terms close the router group while mm1 f1 runs
    for k in range(KD):
        for th in range(TH):
            nc.tensor.matmul(pcu[th][0:E, :], lhsT=cg_sb[k][:, 0:E],
                             rhs=dx16[k][:, TS[th]], start=False, stop=False)
    zrow = consts.tile([1, CGW], BF16, tag="zrow")
    nc.vector.memset(zrow, 0.0)
    for th in range(TH):  # full-width zero matmul closes the whole group
        nc.tensor.matmul(pcu[th][:, 0:E], lhsT=zrow, rhs=ones_row[:, 0:E],
                         start=False, stop=True)

    # top-1 mask without transposes (vector+gpsimd while PE streams mm1):
    #   lgb = (logits + gate_b) + x@dg16 ; mx8 = partition-max(lgb)
    #   mask8 = (lgb == mx8) ; mask32 = REP @ mask8 (PE, r-major) ; um = u*mask
    for th in range(TH):
        nc.vector.tensor_copy(lgc[th], pcu[th][64:72, :])
        nc.vector.scalar_tensor_tensor(lgb[th], pcu[th][0:E, :],
                                       gateb_col[0:E, :], lgc[th], Add, Add)
    for th in range(TH):
        nc.gpsimd.partition_all_reduce(mx8[th], lgb[th], channels=E,
                                       reduce_op=bass.bass_isa.ReduceOp.max)
    for th in range(TH):
        nc.vector.tensor_tensor(mask8[th], lgb[th], mx8[th], IsEq)

    for f in range(1, 6):
        kloop(f)
    for th in range(TH):  # PE reaches this ~20us after the mask chain started
        mkp = bps.tile([P, 512], F32, tag="pbig", name=f"mkp{th}")
        nc.tensor.matmul(mkp[0:ER, :], lhsT=repm_sb, rhs=mask8[th],
                         start=True, stop=True)
        mk32 = rwk.tile([ER, 512], BF16, tag=f"mk32_{th}", name=f"mk32_{th}")
        nc.vector.tensor_copy(mk32, mkp[0:ER, :])
        nc.vector.tensor_mul(um16[th], pcu[th][32:64, :], mk32)
    nlora = 0
    for f in range(6, KF):
        kloop(f)
        if f >= 8:  # 1-2 loras per kloop; lora(j) needs act(j) (j <= f-2)
            take = 2 if (f % 2 == 0) else 1
            while take > 0 and nlora < KF and nlora <= f - 2:
                lora(nlora)
                nlora += 1
                take -= 1
    while nlora < KF:
        lora(nlora)
        nlora += 1

    # ---------- matmul 2: outT = wo @ inter + wo_b (bf16 out DMA) ----------
    for d in range(KD):
        ps = [bps.tile([P, 512], F32, tag="pbig", name=f"p2_{d}_{th}")
              for th in range(TH)]
        for kf in range(KF):
            for th in range(TH):
                nc.tensor.matmul(ps[th],
                                 lhsT=wo_tiles[d][:, kf * P:(kf + 1) * P],
                                 rhs=inter_sb[kf][:, TS[th]],
                                 start=(kf == 0), stop=(kf == KF - 1))
        for th in range(TH):
            osb = outp.tile([P, 512], BF16, tag="osb")
            nc.vector.tensor_scalar(osb, ps[th], wob_sb[:, d:d + 1], None, Add)
            nc.sync.dma_start(out=io["outT"][d * P:(d + 1) * P, TS[th]],
                              in_=osb)


_CACHED_NC = None


def build_nc():
    global _CACHED_NC
    if _CACHED_NC is not None:
        return _CACHED_NC
    nc = bacc.Bacc("TRN2", target_bir_lowering=False, debug=False,
                   enable_asserts=False, num_devices=NCORES)
    decls = [
        ("head", [P, KD * CGW + ER + N], BF16, False),
        ("xT", [P, (KD - 1) * N], BF16, False),
        ("dxT", [P, KD * N], BF16, False),
        ("biases", [P, 41], F32, False),
        ("bT", [ER, F], BF16, False),
        ("wiB", [KF, P, KD * P], BF16, False),
        ("woTt", [KD, P, F], BF16, False),
        ("outT", [D, N], BF16, True),
    ]
    io = {}
    for name, shape, dt_, is_out in decls:
        io[name] = nc.dram_tensor(
            name, shape, dt_, kind="ExternalOutput" if is_out else "ExternalInput"
        ).ap()
    with tile.TileContext(nc) as tc:
        with ExitStack() as ctx:
            _emit(ctx, tc, io)
    nc.compile()
    _CACHED_NC = nc
    return nc


def make_in_maps(inputs: dict) -> list[dict]:
    f32 = np.float32
    x = np.ascontiguousarray(np.asarray(inputs["hidden_states"], f32).reshape(NT, D))
    gT = np.asarray(inputs["gate_W"], f32).T                                # [D, 8]
    gT16 = gT.astype(BF)
    dgT16 = (gT - gT16.astype(f32)).astype(BF)
    a2T16 = (np.asarray(inputs["lora_A"], f32)
             .transpose(1, 0, 2).reshape(ER, D).T.astype(BF))               # [D, 32] r-major
    cg = np.concatenate(
        [gT16, np.zeros((D, 24), BF), a2T16, dgT16], axis=1)                # [D, 72]
    cgh = np.ascontiguousarray(
        cg.reshape(KD, P, CGW).transpose(1, 0, 2).reshape(P, KD * CGW))
    biases = np.zeros((P, 41), f32)
    biases[:, 0:KF] = np.asarray(inputs["wi_b"], f32).reshape(KF, P).T
    biases[:, KF:KF + KD] = np.asarray(inputs["wo_b"], f32).reshape(KD, P).T
    biases[0:E, 40] = np.asarray(inputs["gate_b"], f32)
    repm = np.zeros((P, ER), BF)
    for e in range(E):
        repm[e, np.arange(R) * E + e] = 1
    bT = np.ascontiguousarray(
        np.asarray(inputs["lora_B"], f32).transpose(2, 0, 1).reshape(ER, F).astype(BF))
    wiT = np.asarray(inputs["wi_W"], f32).T.astype(BF)                      # [D, F]
    wiB = np.ascontiguousarray(
        wiT.reshape(KD, P, KF, P).transpose(2, 1, 0, 3).reshape(KF, P, KD * P))
    woT = np.asarray(inputs["wo_W"], f32).T.astype(BF)                      # [F, D]
    woTt = np.ascontiguousarray(
        woT.reshape(KF, P, KD, P).transpose(2, 1, 0, 3).reshape(KD, P, F))

    in_maps = []
    for c in range(NCORES):
        xc32 = x[c * N:(c + 1) * N].T                                       # [D, N]
        xc = xc32.astype(BF)
        dxc = (xc32 - xc.astype(f32)).astype(BF)
        xh = xc.reshape(KD, P, N).transpose(1, 0, 2).reshape(P, KD * N)
        dxh = np.ascontiguousarray(
            dxc.reshape(KD, P, N).transpose(1, 0, 2).reshape(P, KD * N))
        head = np.ascontiguousarray(
            np.concatenate([cgh, repm, xh[:, 0:N]], axis=1))
        in_maps.append({
            "head": head, "xT": np.ascontiguousarray(xh[:, N:]), "dxT": dxh,
            "biases": biases, "bT": bT, "wiB": wiB, "woTt": woTt,
        })
    return in_maps


def kernel(**inputs) -> np.ndarray:
    nc = build_nc()
    in_maps = make_in_maps(inputs)
    res = run_bass_kernel_spmd(nc, in_maps, core_ids=list(range(NCORES)))
    out = np.empty((NT, D), np.float32)
    for c in range(NCORES):
        out[c * N:(c + 1) * N] = res.results[c]["outT"].astype(np.float32).T
    return out.reshape(B, S, D)


# revision 71
# speedup vs baseline: 1.0075x; 1.0075x over previous
"""MoE block (top-1 routing, shared FFN + per-expert LoRA) on 8 TRN2 NeuronCores.

Strategy: data-parallel over the 8192 tokens (1024 tokens/core), weights
replicated. The reference's dense-then-mask expert loop collapses to:

    logits = x @ gate_W.T + gate_b ; mask8 = (logits == rowmax(logits))
    u      = x @ A_cat.T                 [32, N]   (r-major rows: r*8+e)
    u_m    = u * mask8-replicated-4x
    inter  = relu(x @ wi_W.T + Bcat.T @ u_m + wi_b)   (bf16 matmuls)
    out    = inter @ wo_W.T + wo_b

All in transposed (feature-major) layout on chip; the host pre-tiles every
tensor into flat [128, *] layouts so each DMA is a cheap 2D descriptor set,
and re-transposes the output. Logits are fp32-accurate via 3 bf16 terms
(x16@g16 + dx16@g16 + x16@dg16) so routing matches the fp32 argmax; the
top-1 mask is built without PE transposes (gpsimd cross-partition max +
vector equality + a tiny PE replicate matmul).

Schedule: the PE runs one continuous in-order stream — warm-up fillers
(hold the clock-gate/p-state up while DMAs land), router A-terms chasing
the x chunks, then mm1's k-loops paced by the wi stream. The lora term is
decoupled from mm1's PSUM accumulation (base = x@wiT + wi_b goes to SBUF
in bf16; standalone lora matmuls + vector add/relu finalize inter later),
so the router's mask-chain latency never blocks the PE pipeline. All big
DMAs ride the sync hw-DGE ring ordered by first use; outputs (bf16) drain
on the same ring behind the weight loads.
"""

import numpy as np
import ml_dtypes
from contextlib import ExitStack

import concourse.bass as bass
import concourse.tile as tile
from concourse import bacc, mybir
from concourse.bass_utils import run_bass_kernel_spmd

F32 = mybir.dt.float32
BF16 = mybir.dt.bfloat16
BF = ml_dtypes.bfloat16

B, S, D, F, E, R = 4, 2048, 1024, 4096, 8, 4
NCORES = 8
NT = B * S          # 8192 tokens total
N = NT // NCORES    # 1024 tokens per core
ER = E * R          # 32 lora rows (r-major: row r*8+e)
KD = D // 128       # 8 contraction tiles over D
KF = F // 128       # 32 contraction tiles over F
TH = N // 512       # 2 token halves (matmul moving dim)
P = 128
# router stationary cols: [g16(0:8) | zeros(8:32) | Acat16(32:64) | dg16(64:72)]
# -> psum rows: logits at 0:8 (x@g16 + dx@g16), u at 32:64, x@dg16 at 64:72;
# all reads land on quadrant-aligned partition offsets (0/32/64).
CGW = 72
NWARM = 10          # PE warm-up matmuls (keep HAM clock up during DMA-in)
NFILL = 4           # PE fillers between router A-terms and mm1

Relu = mybir.ActivationFunctionType.Relu
Add = mybir.AluOpType.add
IsEq = mybir.AluOpType.is_equal


def _emit(ctx: ExitStack, tc: tile.TileContext, io: dict):
    nc = tc.nc

    consts = ctx.enter_context(tc.tile_pool(name="consts", bufs=1))
    xpool = ctx.enter_context(tc.tile_pool(name="xpool", bufs=1))
    wipool = ctx.enter_context(tc.tile_pool(name="wipool", bufs=1))
    ipool = ctx.enter_context(tc.tile_pool(name="ipool", bufs=1))
    wop = ctx.enter_context(tc.tile_pool(name="wop", bufs=2))
    rwk = ctx.enter_context(tc.tile_pool(name="rwk", bufs=1))
    outp = ctx.enter_context(tc.tile_pool(name="outp", bufs=3))
    sps = ctx.enter_context(tc.tile_pool(name="sps", bufs=1, space="PSUM"))
    bps = ctx.enter_context(tc.tile_pool(name="bps", bufs=6, space="PSUM"))

    TS = [slice(th * 512, (th + 1) * 512) for th in range(TH)]

    # ---------- input DMAs, all on the sync (hw-DGE) queue, in arrival-
    # priority order: router weights, x (4 chunks for fine-grained deps),
    # lora B / biases, wi per f-tile, wo per d-tile.
    # head = cg | repm(row-padded) | x chunk 0, fetched as ONE first DMA
    HCG = KD * CGW
    head_t = consts.tile([P, HCG + ER + N], BF16, tag="head")
    nc.sync.dma_start(out=head_t, in_=io["head"])
    xt = xpool.tile([P, (KD - 1) * N], BF16, tag="x")
    dxt = xpool.tile([P, KD * N], BF16, tag="dx")
    for j in range(KD - 1):  # one chunk per k-tile: router starts ASAP
        sl = slice(j * N, (j + 1) * N)
        nc.sync.dma_start(out=xt[:, sl], in_=io["xT"][:, sl])
    wiB = []
    for f in range(KF):
        t = wipool.tile([P, KD * P], BF16, tag=f"wi{f}")
        wiB.append(t)
    for f in range(3):
        nc.sync.dma_start(out=wiB[f], in_=io["wiB"][f])
    biases_sb = consts.tile([P, 41], F32, tag="biases")
    nc.sync.dma_start(out=biases_sb, in_=io["biases"])
    # dx is only needed once mm1 f0's k-loop is already running
    for j in range(4):
        sl = slice(j * 2 * N, (j + 1) * 2 * N)
        nc.sync.dma_start(out=dxt[:, sl], in_=io["dxT"][:, sl])
    bcat_sb = consts.tile([ER, F], BF16, tag="bc")
    nc.sync.dma_start(out=bcat_sb, in_=io["bT"])
    for f in range(3, KF):
        nc.sync.dma_start(out=wiB[f], in_=io["wiB"][f])
    repm_sb = head_t[0:E, HCG:HCG + ER]
    wo_tiles = []
    for d in range(KD):
        t = wop.tile([P, F], BF16, tag="wo", name=f"wo{d}")
        nc.sync.dma_start(out=t, in_=io["woTt"][d])
        wo_tiles.append(t)

    cg_sb = [head_t[:, k * CGW:(k + 1) * CGW] for k in range(KD)]
    x16 = [head_t[:, HCG + ER:HCG + ER + N]] + \
          [xt[:, k * N:(k + 1) * N] for k in range(KD - 1)]
    dx16 = [dxt[:, k * N:(k + 1) * N] for k in range(KD)]
    wib_sb = biases_sb[:, 0:KF]
    wob_sb = biases_sb[:, KF:KF + KD]
    gateb_col = biases_sb[:, 40:41]

    # ---------- PE warm-up: results never consumed; spins the PE from
    # engine-boot, holding the HAM clock gate open while the input DMAs land.
    warm_src = consts.tile([P, 512], BF16, tag="warm")
    nc.gpsimd.memset(warm_src, 1.0)
    for w in range(NWARM):
        pw = bps.tile([P, 512], F32, tag="pbig", name=f"pw{w}")
        nc.tensor.matmul(pw, lhsT=warm_src[:, 0:P], rhs=warm_src,
                         start=True, stop=True)

    # ---------- router + lora projection (fp32-accurate logits via 3 bf16
    # terms: x16@g16 + dx16@g16 + x16@dg16; u needs no correction).
    # A-terms chase the x chunks (with fillers bridging DMA-arrival waits);
    # the dx B-terms are deferred to overlap with mm1's first k-loop, since
    # only the bcat stop (not the k-loops) depends on the router.
    pcu = [sps.tile([CGW, 512], F32, tag=f"pcu{th}", name=f"pcu{th}")
           for th in range(TH)]
    nfil = 0
    for k in range(KD):
        for th in range(TH):
            nc.tensor.matmul(pcu[th], lhsT=cg_sb[k], rhs=x16[k][:, TS[th]],
                             start=(k == 0), stop=False)
        # fillers bridge the DMA arrival rate (slower during DGE ramp-up)
        for _ in range(2 if 0 < k < 4 else 1):
            pw = bps.tile([P, 512], F32, tag="pbig", name=f"pf{nfil}")
            nc.tensor.matmul(pw, lhsT=warm_src[:, 0:P], rhs=warm_src,
                             start=True, stop=True)
            nfil += 1
    for w in range(NFILL):
        pw = bps.tile([P, 512], F32, tag="pbig", name=f"pf{nfil + w}")
        nc.tensor.matmul(pw, lhsT=warm_src[:, 0:P], rhs=warm_src,
                         start=True, stop=True)

    um16, lgb, mask8 = [], [], []
    for th in range(TH):
        lgb.append(rwk.tile([E, 512], F32, tag=f"lgb{th}", name=f"lgb{th}"))
        mask8.append(rwk.tile([E, 512], BF16, tag=f"mk{th}", name=f"mk{th}"))
        um16.append(rwk.tile([ER, 512], BF16, tag=f"um{th}", name=f"um{th}"))
    mx8 = [rwk.tile([E, 512], F32, tag=f"mx8_{th}", name=f"mx8_{th}")
           for th in range(TH)]
    lgc = [rwk.tile([E, 512], F32, tag=f"lgc{th}", name=f"lgc{th}")
           for th in range(TH)]

    inter_sb = [ipool.tile([P, N], BF16, tag=f"inter{f}", name=f"inter{f}")
                for f in range(KF)]

    # mm1 is decoupled from the router: each f writes base = x@wiT + wi_b to
    # SBUF (bf16, Identity act). The lora term is added later by standalone
    # matmuls + a vector add/relu pass, so the router's mask-chain latency
    # never blocks the PE pipeline.
    def kloop(f):
        ps = [bps.tile([P, 512], F32, tag="pbig", name=f"p1_{f}_{th}")
              for th in range(TH)]
        for k in range(KD):
            for th in range(TH):
                nc.tensor.matmul(ps[th], lhsT=wiB[f][:, k * P:(k + 1) * P],
                                 rhs=x16[k][:, TS[th]],
                                 start=(k == 0), stop=(k == KD - 1))
        for th in range(TH):
            nc.scalar.activation(inter_sb[f][:, TS[th]], ps[th],
                                 mybir.ActivationFunctionType.Identity,
                                 bias=wib_sb[:, f:f + 1])
        return ps

    def lora(f):
        # vector does the PSUM-reading add; gpsimd (no PSUM access) the relu
        for th in range(TH):
            lp = bps.tile([P, 512], F32, tag="pbig", name=f"lp{f}_{th}")
            nc.tensor.matmul(lp[0:P, :], lhsT=bcat_sb[:, f * P:(f + 1) * P],
                             rhs=um16[th], start=True, stop=True)
            sl = inter_sb[f][:, TS[th]]
            nc.vector.scalar_tensor_tensor(sl, lp, 0.0, sl, Add, Add)
            nc.vector.tensor_scalar(sl, sl, 0.0, None, mybir.AluOpType.max)

    kloop(0)
    # dx B-terms close the router group while mm1 f1 runs. Full 72-wide
    # stationary: dx@A refines u and dx@dg refines the correction (both
    # strictly more accurate), and B(7) is a legal full-width group stop.
    for k in range(KD):
        for th in range(TH):
            nc.tensor.matmul(pcu[th], lhsT=cg_sb[k],
                             rhs=dx16[k][:, TS[th]], start=False,
                             stop=(k == KD - 1))

    # top-1 mask without transposes (vector+gpsimd while PE streams mm1):
    #   lgb = (logits + gate_b) + x@dg16 ; mx8 = partition-max(lgb)
    #   mask8 = (lgb == mx8) ; mask32 = REP @ mask8 (PE, r-major) ; um = u*mask
    for th in range(TH):
        nc.vector.tensor_copy(lgc[th], pcu[th][64:72, :])
        nc.vector.scalar_tensor_tensor(lgb[th], pcu[th][0:E, :],
                                       gateb_col[0:E, :], lgc[th], Add, Add)
    for th in range(TH):
        nc.gpsimd.partition_all_reduce(mx8[th], lgb[th], channels=E,
                                       reduce_op=bass.bass_isa.ReduceOp.max)
    for th in range(TH):
        nc.vector.tensor_tensor(mask8[th], lgb[th], mx8[th], IsEq)

    for f in range(1, 6):
        kloop(f)
    for th in range(TH):  # PE reaches this ~20us after the mask chain started
        mkp = bps.tile([P, 512], F32, tag="pbig", name=f"mkp{th}")
        nc.tensor.matmul(mkp[0:ER, :], lhsT=repm_sb, rhs=mask8[th],
                         start=True, stop=True)
        mk32 = rwk.tile([ER, 512], BF16, tag=f"mk32_{th}", name=f"mk32_{th}")
        nc.vector.tensor_copy(mk32, mkp[0:ER, :])
        nc.vector.tensor_mul(um16[th], pcu[th][32:64, :], mk32)
    nlora = 0
    for f in range(6, KF):
        kloop(f)
        if f >= 8:  # 1-2 loras per kloop; lora(j) needs act(j) (j <= f-2)
            take = 2 if (f % 2 == 0) else 1
            while take > 0 and nlora < KF and nlora <= f - 2:
                lora(nlora)
                nlora += 1
                take -= 1
    while nlora < KF:
        lora(nlora)
        nlora += 1

    # ---------- matmul 2: outT = wo @ inter + wo_b (bf16 out DMA) ----------
    for d in range(KD):
        ps = [bps.tile([P, 512], F32, tag="pbig", name=f"p2_{d}_{th}")
              for th in range(TH)]
        for kf in range(KF):
            for th in range(TH):
                nc.tensor.matmul(ps[th],
                                 lhsT=wo_tiles[d][:, kf * P:(kf + 1) * P],
                                 rhs=inter_sb[kf][:, TS[th]],
                                 start=(kf == 0), stop=(kf == KF - 1))
        for th in range(TH):
            osb = outp.tile([P, 512], BF16, tag="osb")
            nc.vector.tensor_scalar(osb, ps[th], wob_sb[:, d:d + 1], None, Add)
            nc.sync.dma_start(out=io["outT"][d * P:(d + 1) * P, TS[th]],
                              in_=osb)


_CACHED_NC = None


def build_nc():
    global _CACHED_NC
    if _CACHED_NC is not None:
        return _CACHED_NC
    nc = bacc.Bacc("TRN2", target_bir_lowering=False, debug=False,
                   enable_asserts=False, num_devices=NCORES)
    decls = [
        ("head", [P, KD * CGW + ER + N], BF16, False),
        ("xT", [P, (KD - 1) * N], BF16, False),
        ("dxT", [P, KD * N], BF16, False),
        ("biases", [P, 41], F32, False),
        ("bT", [ER, F], BF16, False),
        ("wiB", [KF, P, KD * P], BF16, False),
        ("woTt", [KD, P, F], BF16, False),
        ("outT", [D, N], BF16, True),
    ]
    io = {}
    for name, shape, dt_, is_out in decls:
        io[name] = nc.dram_tensor(
            name, shape, dt_, kind="ExternalOutput" if is_out else "ExternalInput"
        ).ap()
    with tile.TileContext(nc) as tc:
        with ExitStack() as ctx:
            _emit(ctx, tc, io)
    nc.compile()
    _CACHED_NC = nc
    return nc


def make_in_maps(inputs: dict) -> list[dict]:
    f32 = np.float32
    x = np.ascontiguousarray(np.asarray(inputs["hidden_states"], f32).reshape(NT, D))
    gT = np.asarray(inputs["gate_W"], f32).T                                # [D, 8]
    gT16 = gT.astype(BF)
    dgT16 = (gT - gT16.astype(f32)).astype(BF)
    a2T16 = (np.asarray(inputs["lora_A"], f32)
             .transpose(1, 0, 2).reshape(ER, D).T.astype(BF))               # [D, 32] r-major
    cg = np.concatenate(
        [gT16, np.zeros((D, 24), BF), a2T16, dgT16], axis=1)                # [D, 72]
    cgh = np.ascontiguousarray(
        cg.reshape(KD, P, CGW).transpose(1, 0, 2).reshape(P, KD * CGW))
    biases = np.zeros((P, 41), f32)
    biases[:, 0:KF] = np.asarray(inputs["wi_b"], f32).reshape(KF, P).T
    biases[:, KF:KF + KD] = np.asarray(inputs["wo_b"], f32).reshape(KD, P).T
    biases[0:E, 40] = np.asarray(inputs["gate_b"], f32)
    repm = np.zeros((P, ER), BF)
    for e in range(E):
        repm[e, np.arange(R) * E + e] = 1
    bT = np.ascontiguousarray(
        np.asarray(inputs["lora_B"], f32).transpose(2, 0, 1).reshape(ER, F).astype(BF))
    wiT = np.asarray(inputs["wi_W"], f32).T.astype(BF)                      # [D, F]
    wiB = np.ascontiguousarray(
        wiT.reshape(KD, P, KF, P).transpose(2, 1, 0, 3).reshape(KF, P, KD * P))
    woT = np.asarray(inputs["wo_W"], f32).T.astype(BF)                      # [F, D]
    woTt = np.ascontiguousarray(
        woT.reshape(KF, P, KD, P).transpose(2, 1, 0, 3).reshape(KD, P, F))

    in_maps = []
    for c in range(NCORES):
        xc32 = x[c * N:(c + 1) * N].T                                       # [D, N]
        xc = xc32.astype(BF)
        dxc = (xc32 - xc.astype(f32)).astype(BF)
        xh = xc.reshape(KD, P, N).transpose(1, 0, 2).reshape(P, KD * N)
        dxh = np.ascontiguousarray(
            dxc.reshape(KD, P, N).transpose(1, 0, 2).reshape(P, KD * N))
        head = np.ascontiguousarray(
            np.concatenate([cgh, repm, xh[:, 0:N]], axis=1))
        in_maps.append({
            "head": head, "xT": np.ascontiguousarray(xh[:, N:]), "dxT": dxh,
            "biases": biases, "bT": bT, "wiB": wiB, "woTt": woTt,
        })
    return in_maps


def kernel(**inputs) -> np.ndarray:
    nc = build_nc()
    in_maps = make_in_maps(inputs)
    res = run_bass_kernel_spmd(nc, in_maps, core_ids=list(range(NCORES)))
    out = np.empty((NT, D), np.float32)
    for c in range(NCORES):
        out[c * N:(c + 1) * N] = res.results[c]["outT"].astype(np.float32).T
    return out.reshape(B, S, D)
